# revision 24
# baseline (speedup 1.0000x reference)
"""Trainium2 Bass kernel for BasicRecurrentEntityEncoder.

Math (reference):
  word_emb = embedding_matrix[prgrph]                  # [B,S,L,E] gather
  sent_enc = sum_L word_emb                            # [B,S,E]
  enc = (sent_enc @ fc1_w + b1) @ fc2_w + b2           # [B,S,D]
  scan over S steps of an entity-cell update on h [B,K,D] with l2-norm
  and per-batch sentence mask.

Strategy (8 cores, data-parallel over batch, 8 paragraphs/core):
  - The embedding gather (bf16, host-converted table), bag-of-words
    sums (bf16 PE matmuls vs block-ones), and the folded FC pipeline
    run CHUNK-WISE (16 tiles = 64 sentences = 8 scan steps per chunk),
    interleaved with the scan so the gather's DMA descriptor
    generation hides under scan compute.
  - Scan: state H transposed [D=128 partitions, b*k free], split into
    2 paragraph groups; partition reductions/broadcasts on the PE
    (ones-vector fp16 matmuls); l2-norm via exp(-0.5*ln(ss+eps)).
  - gate: sigmoid and the sentence mask fold into
    1/((exp(-logit)+1)*rminv) with rminv in {1, 1e20}; the gate row is
    cast to fp16 and broadcast down partitions via a rank-1 PE matmul.
  - A custom Bacc pass pins every activation to the
    natural_log_exp_and_others table: zero ACT table switches.
  - U.T@h uses rhs=(h+kt) in fp16 with kvsw' = kV + sW - U.T@kt
    precomputed per chunk.
"""

import numpy as np
import ml_dtypes

import bass_rust as _bass_rust
import concourse.bacc as bacc
import concourse.bass as bass
import concourse.mybir as mybir
import concourse.tile as tile
from concourse.bass import AP
from concourse.bass_utils import run_bass_kernel_spmd
from concourse.hw_specs import get_activation_tables

F32 = mybir.dt.float32
F16 = mybir.dt.float16
BF16 = mybir.dt.bfloat16
I32 = mybir.dt.int32
AF = mybir.ActivationFunctionType
OP = mybir.AluOpType

B, S, L = 64, 128, 32
V_SZ, E, D, K = 50000, 256, 128, 20
NCORES = 8
BL = B // NCORES          # 8 paragraphs per core
BK = BL * K               # 160 (b,k) columns per core
NSENT = S * BL            # 1024 sentences per core
NTOK = NSENT * L          # 32768 tokens per core
NTILES = NTOK // 128      # 256 gather tiles of 128 tokens
TPC = 16                  # gather tiles per chunk (= 8 scan steps)
SPC = TPC * 4 // BL       # scan steps per chunk (8)
EPS = 1e-6                # l2-norm epsilon (fp16-safe: rsqrt <= 1e3)
MINV = 1e20               # masked-row multiplier for the gate denominator

GROUPS = 2                # paragraph groups in the scan
GBL = BL // GROUPS        # paragraphs per group
GW = GBL * K              # columns per group

_FUNCS = {"ln", "exp", "relu", "square", "identity", "copy"}


class _Bacc(bacc.Bacc):
    """Bacc with the ACT-table pass pinned to natural_log_exp_and_others."""

    def insert_act_table_loads(self):
        tables = []
        for name, funcs in get_activation_tables(self.m.arch).items():
            if name != "natural_log_exp_and_others":
                funcs = {f for f in funcs if f.name.lower() not in _FUNCS}
            tables.append((name, funcs))
        _bass_rust.insert_act_table_loads(self, tables)


def _bk3(ap2d):
    """[P, nb*K] slice -> [P, nb, K] view (b outer, k inner)."""
    return ap2d.rearrange("p (b k) -> p b k", k=K)


def _j3(ap2d):
    """[P, nb] slice -> [P, nb, K] view broadcasting along the entity dim."""
    return AP(ap2d.tensor, ap2d.offset, list(ap2d.ap) + [[0, K]])


def build_bass(n_steps=S, debug=False):
    nc = _Bacc()

    embc = nc.dram_tensor("embc", [V_SZ, E], BF16, kind="ExternalInput")
    idx = nc.dram_tensor("idx", [128, NTILES], I32, kind="ExternalInput")
    kt = nc.dram_tensor("kt", [D, BK], F32, kind="ExternalInput")
    kt16 = nc.dram_tensor("kt16", [D, BK], F16, kind="ExternalInput")
    u16 = nc.dram_tensor("u16", [D, D], F16, kind="ExternalInput")
    id16 = nc.dram_tensor("id16", [D, D], F16, kind="ExternalInput")
    ocol16 = nc.dram_tensor("ocol16", [D, 1], F16, kind="ExternalInput")
    orow16 = nc.dram_tensor("orow16", [1, D], F16, kind="ExternalInput")
    ones4 = nc.dram_tensor("ones4", [128, 4], BF16, kind="ExternalInput")
    v_w = nc.dram_tensor("v_w", [D, D], F32, kind="ExternalInput")
    w_w = nc.dram_tensor("w_w", [D, D], F32, kind="ExternalInput")
    f1t = nc.dram_tensor("f1t", [D, E], F32, kind="ExternalInput")
    f2w = nc.dram_tensor("f2w", [D, D], F32, kind="ExternalInput")
    f1b = nc.dram_tensor("f1b", [D, 1], F32, kind="ExternalInput")
    f2b = nc.dram_tensor("f2b", [D, 1], F32, kind="ExternalInput")
    rminv = nc.dram_tensor("rminv", [1, NSENT], F32, kind="ExternalInput")
    out = nc.dram_tensor("out", [D, BK], F32, kind="ExternalOutput")
    if debug:
        d_enc = nc.dram_tensor("d_enc", [D, NSENT], F32, kind="ExternalOutput")
        d_kvsw = nc.dram_tensor("d_kvsw", [D, 2 * BK], F16,
                                kind="ExternalOutput")
        d_sent0 = nc.dram_tensor("d_sent0", [128, NSENT], F32,
                                 kind="ExternalOutput")

    n_chunks = (n_steps + SPC - 1) // SPC

    with tile.TileContext(nc) as tc:
        with tc.tile_pool(name="persist", bufs=1) as pp:
            idx_sb = pp.tile([128, NTILES], I32, tag="idx_sb")
            kt_sb = pp.tile([D, BK], F32, tag="kt_sb")
            kt16_sb = pp.tile([D, BK], F16, tag="kt16_sb")
            u16_sb = pp.tile([D, D], F16, tag="u16_sb")
            id16_sb = pp.tile([D, D], F16, tag="id16_sb")
            ocol_sb = pp.tile([D, 1], F16, tag="ocol_sb")
            orow_sb = pp.tile([1, D], F16, tag="orow_sb")
            ones4_sb = pp.tile([128, 4], BF16, tag="ones4_sb")
            v_sb = pp.tile([D, D], F32, tag="v_sb")
            w_sb = pp.tile([D, D], F32, tag="w_sb")
            f1t_sb = pp.tile([D, E], F32, tag="f1t_sb")
            f2w_sb = pp.tile([D, D], F32, tag="f2w_sb")
            f1b_sb = pp.tile([D, 1], F32, tag="f1b_sb")
            f2b_sb = pp.tile([D, 1], F32, tag="f2b_sb")
            rminv_sb = pp.tile([1, NSENT], F32, tag="rminv_sb")
            sent0 = pp.tile([128, NSENT], F32, tag="sent0")
            sent1 = pp.tile([128, NSENT], F32, tag="sent1")
            wc0 = pp.tile([D, D], F32, tag="wc0")
            wc1 = pp.tile([D, D], F32, tag="wc1")
            bc_sb = pp.tile([D, 1], F32, tag="bc_sb")
            enc_sb = pp.tile([D, NSENT], F32, tag="enc_sb")
            enc16_sb = pp.tile([D, NSENT], F16, tag="enc16_sb")
            encw_sb = pp.tile([D, NSENT], F32, tag="encw_sb")
            kv_sb = pp.tile([D, BK], F32, tag="kv_sb")
            kvu_sb = pp.tile([D, BK], F32, tag="kvu_sb")
            kvsw16 = pp.tile([D, S * BK], F16, tag="kvsw16")
            eps_sb = pp.tile([1, 1], F32, tag="eps_sb")
            nc.vector.memset(eps_sb[:], EPS)

            for sb, dr in [(idx_sb, idx), (kt_sb, kt), (kt16_sb, kt16),
                           (u16_sb, u16), (id16_sb, id16), (ocol_sb, ocol16),
                           (orow_sb, orow16), (ones4_sb, ones4),
                           (v_sb, v_w), (w_sb, w_w), (f1t_sb, f1t),
                           (f2w_sb, f2w), (f1b_sb, f1b), (f2b_sb, f2b),
                           (rminv_sb, rminv)]:
                nc.sync.dma_start(out=sb[:], in_=dr[:])

            with tc.tile_pool(name="gpool", bufs=4) as gp, \
                 tc.tile_pool(name="gps", bufs=2, space="PSUM") as gps, \
                 tc.tile_pool(name="eps_p", bufs=1, space="PSUM") as epp, \
                 tc.tile_pool(name="scan", bufs=2) as sp, \
                 tc.tile_pool(name="scanp", bufs=1, space="PSUM") as pps, \
                 tc.tile_pool(name="scanb", bufs=1, space="PSUM") as ppb:

                # ---- setup: folded FC weights, kv, ukt (no gather dep) ----
                ps_s = epp.tile([D, D], F32, tag="encp")
                nc.tensor.matmul(out=ps_s[:], lhsT=f1t_sb[:, 0:D],
                                 rhs=f2w_sb[:], start=True, stop=True)
                nc.vector.tensor_copy(out=wc0[:], in_=ps_s[:])
                ps_s = epp.tile([D, D], F32, tag="encp")
                nc.tensor.matmul(out=ps_s[:], lhsT=f1t_sb[:, D:E],
                                 rhs=f2w_sb[:], start=True, stop=True)
                nc.vector.tensor_copy(out=wc1[:], in_=ps_s[:])
                ps_s = epp.tile([D, 1], F32, tag="encp")
                nc.tensor.matmul(out=ps_s[:], lhsT=f2w_sb[:], rhs=f1b_sb[:],
                                 start=True, stop=True)
                nc.vector.tensor_add(out=bc_sb[:], in0=ps_s[:], in1=f2b_sb[:])
                ps_s = epp.tile([D, BK], F32, tag="encp")
                nc.tensor.matmul(out=ps_s[:], lhsT=v_sb[:], rhs=kt_sb[:],
                                 start=True, stop=True)
                nc.vector.tensor_copy(out=kv_sb[:], in_=ps_s[:])
                ps_s = epp.tile([D, BK], F32, tag="encp")
                nc.tensor.matmul(out=ps_s[:], lhsT=u16_sb[:], rhs=kt16_sb[:],
                                 start=True, stop=True)
                nc.vector.tensor_sub(out=kvu_sb[:], in0=kv_sb[:], in1=ps_s[:])

                # scan state
                h = []
                for g in range(GROUPS):
                    hg = sp.tile([D, GW], F32, tag=f"H{g}")
                    nc.vector.memset(hg[:], 0.0)
                    h.append(hg)

                gats, psGs = {}, {}

                def emit_gather_pair(c, j0):
                    # 2 gather tiles + their bag-of-words sums
                    if j0 == 0:
                        gats[c] = gp.tile([128, TPC, E], BF16, tag="gat",
                                          name=f"gat{c}")
                        psGs[c] = gps.tile([128, 2, 4 * TPC], F32, tag="gsum",
                                           name=f"psG{c}")
                    gat, psG = gats[c], psGs[c]
                    for j in (j0, j0 + 1):
                        ti = c * TPC + j
                        nc.gpsimd.indirect_dma_start(
                            out=gat[:, j, :], out_offset=None, in_=embc[:],
                            in_offset=bass.IndirectOffsetOnAxis(
                                ap=idx_sb[:, ti:ti + 1], axis=0),
                        )
                    for j in (j0, j0 + 1):
                        nc.tensor.matmul(out=psG[:, 0, 4 * j:4 * j + 4],
                                         lhsT=gat[:, j, 0:D], rhs=ones4_sb[:],
                                         start=True, stop=True)
                        nc.tensor.matmul(out=psG[:, 1, 4 * j:4 * j + 4],
                                         lhsT=gat[:, j, D:E], rhs=ones4_sb[:],
                                         start=True, stop=True)

                def emit_chunk_fc(c):
                    psG = psGs.pop(c)
                    gats.pop(c)
                    snt = slice(c * 4 * TPC, (c + 1) * 4 * TPC)
                    nc.vector.tensor_copy(out=sent0[:, snt], in_=psG[:, 0, :])
                    nc.vector.tensor_copy(out=sent1[:, snt], in_=psG[:, 1, :])

                    # ---- FC chunk: enc, enc16, encw, kvsw' slices ----
                    ps_e = epp.tile([128, 4 * TPC], F32, tag="encp")
                    nc.tensor.matmul(out=ps_e[:], lhsT=wc0[:],
                                     rhs=sent0[:, snt], start=True, stop=False)
                    nc.tensor.matmul(out=ps_e[:], lhsT=wc1[:],
                                     rhs=sent1[:, snt], start=False, stop=True)
                    nc.scalar.activation(out=enc_sb[:, snt], in_=ps_e[:],
                                         func=AF.Identity, bias=bc_sb[:, 0:1])
                    nc.scalar.activation(out=enc16_sb[:, snt],
                                         in_=enc_sb[:, snt], func=AF.Copy)
                    ps_w = epp.tile([128, 4 * TPC], F32, tag="encp")
                    nc.tensor.matmul(out=ps_w[:], lhsT=w_sb[:],
                                     rhs=enc_sb[:, snt], start=True, stop=True)
                    nc.vector.tensor_copy(out=encw_sb[:, snt], in_=ps_w[:])

                    ko = c * SPC * BK
                    kvsw_v = AP(kvsw16.tensor, kvsw16[:].offset + ko,
                                [kvsw16[:].ap[0], [BK, SPC], [K, BL], [1, K]])
                    kvu_v = AP(kvu_sb.tensor, kvu_sb[:].offset,
                               [kvu_sb[:].ap[0], [0, SPC], [K, BL], [1, K]])
                    encw_v = AP(encw_sb.tensor,
                                encw_sb[:].offset + c * SPC * BL,
                                [encw_sb[:].ap[0], [BL, SPC], [1, BL], [0, K]])
                    nc.vector.tensor_tensor(out=kvsw_v, in0=kvu_v, in1=encw_v,
                                            op=OP.add)

                def emit_step(t):
                    nonlocal h
                    if True:
                        csl = [slice(g * GW, (g + 1) * GW)
                               for g in range(GROUPS)]
                        ksl = [slice(t * BK + g * GW, t * BK + (g + 1) * GW)
                               for g in range(GROUPS)]

                        # PE: inject kvsw' (no h dependency; issues early)
                        ps_ht = []
                        for g in range(GROUPS):
                            p = pps.tile([D, GW], F32, tag=f"ht{g}")
                            nc.tensor.matmul(out=p[:], lhsT=id16_sb[:],
                                             rhs=kvsw16[:, ksl[g]],
                                             start=True, stop=False)
                            ps_ht.append(p)

                        # tmp16 = h + kt; prod16 = tmp16 * s (shared tile)
                        tmp16 = []
                        for g in range(GROUPS):
                            tg = sp.tile([D, GW], F16, tag=f"tmp{g}")
                            nc.vector.tensor_add(out=tg[:], in0=h[g][:],
                                                 in1=kt_sb[:, csl[g]])
                            tmp16.append(tg)
                        prod16 = sp.tile([D, BK], F16, tag="prod")
                        for g in range(GROUPS):
                            e16v = _j3(enc16_sb[:, t * BL + g * GBL:
                                                t * BL + (g + 1) * GBL])
                            nc.vector.tensor_tensor(
                                out=_bk3(prod16[:, csl[g]]),
                                in0=_bk3(tmp16[g][:]), in1=e16v, op=OP.mult)

                        # PE: logit row; U.T @ tmp16 onto kvsw'
                        ps_g = pps.tile([1, BK], F32, tag="row")
                        nc.tensor.matmul(out=ps_g[:], lhsT=ocol_sb[:],
                                         rhs=prod16[:], start=True, stop=True)
                        for g in range(GROUPS):
                            nc.tensor.matmul(out=ps_ht[g][:], lhsT=u16_sb[:],
                                             rhs=tmp16[g][:],
                                             start=False, stop=True)

                        # gate row chain (shared across groups)
                        ex = sp.tile([1, BK], F32, tag="ex")
                        nc.scalar.activation(out=ex[:], in_=ps_g[:],
                                             func=AF.Exp, scale=-1.0)
                        ht = []
                        for g in range(GROUPS):
                            hgt = sp.tile([D, GW], F32, tag=f"htld{g}")
                            nc.scalar.activation(out=hgt[:], in_=ps_ht[g][:],
                                                 func=AF.Relu)
                            ht.append(hgt)
                        den = sp.tile([1, BK], F32, tag="den")
                        rv = _j3(rminv_sb[0:1, t * BL:(t + 1) * BL])
                        nc.vector.scalar_tensor_tensor(
                            out=_bk3(den[:]), in0=_bk3(ex[:]), scalar=1.0,
                            in1=rv, op0=OP.add, op1=OP.mult)
                        rden = sp.tile([1, BK], F32, tag="rden")
                        nc.vector.reciprocal_approx_fast(out=rden[:],
                                                         in_=den[:])
                        gm16 = sp.tile([1, BK], F16, tag="gm16")
                        nc.vector.tensor_copy(out=gm16[:], in_=rden[:])

                        # broadcast gate; gh = ht * gate; uu = h + gh
                        gh, uu = [], []
                        for g in range(GROUPS):
                            ps_gb = ppb.tile([D, GW], F32, tag=f"bc{g}")
                            nc.tensor.matmul(out=ps_gb[:], lhsT=orow_sb[:],
                                             rhs=gm16[0:1, csl[g]],
                                             start=True, stop=True)
                            gg = sp.tile([D, GW], F32, tag=f"gh{g}")
                            nc.vector.tensor_mul(out=gg[:], in0=ht[g][:],
                                                 in1=ps_gb[:])
                            gh.append(gg)
                        uu_t = sp.tile([D, BK], F32, tag="uu")
                        for g in range(GROUPS):
                            nc.vector.tensor_add(out=uu_t[:, csl[g]],
                                                 in0=h[g][:], in1=gh[g][:])
                            uu.append(uu_t)

                        # norm: sq16 (one ACT op); ss row; rn; bcast; hn
                        sq16 = sp.tile([D, BK], F16, tag="sq")
                        nc.scalar.activation(out=sq16[:], in_=uu_t[:],
                                             func=AF.Square)
                        ps_ss = pps.tile([1, BK], F32, tag="row")
                        nc.tensor.matmul(out=ps_ss[:], lhsT=ocol_sb[:],
                                         rhs=sq16[:], start=True, stop=True)
                        lg = sp.tile([1, BK], F32, tag="lg")
                        nc.scalar.activation(out=lg[:], in_=ps_ss[:],
                                             func=AF.Ln, bias=eps_sb[:, 0:1])
                        rn16 = sp.tile([1, BK], F16, tag="rn16")
                        nc.scalar.activation(out=rn16[:], in_=lg[:],
                                             func=AF.Exp, scale=-0.5)
                        hn = []
                        for g in range(GROUPS):
                            ps_rn = ppb.tile([D, GW], F32, tag=f"bc{g}")
                            nc.tensor.matmul(out=ps_rn[:], lhsT=orow_sb[:],
                                             rhs=rn16[0:1, csl[g]],
                                             start=True, stop=True)
                            hg = sp.tile([D, GW], F32, tag=f"H{g}")
                            nc.vector.tensor_mul(out=hg[:],
                                                 in0=uu_t[:, csl[g]],
                                                 in1=ps_rn[:])
                            hn.append(hg)
                        h = hn

                PIPE = 2
                for c in range(n_chunks + PIPE):
                    cc = c - PIPE
                    for sl in range(SPC):
                        if c < n_chunks:
                            emit_gather_pair(c, 2 * sl)
                        if c >= PIPE and cc * SPC + sl < n_steps:
                            emit_step(cc * SPC + sl)
                    if c < n_chunks:
                        emit_chunk_fc(c)

                for g in range(GROUPS):
                    nc.sync.dma_start(out=out[:, g * GW:(g + 1) * GW],
                                      in_=h[g][:])
                if debug:
                    nc.sync.dma_start(out=d_enc[:], in_=enc_sb[:])
                    nc.sync.dma_start(out=d_kvsw[:], in_=kvsw16[:, 0:2 * BK])
                    nc.sync.dma_start(out=d_sent0[:], in_=sent0[:])

    if not nc.is_finalized():
        nc.finalize()
    return nc


def _prep_core(c, prgrph, sent_mask, keys):
    pr = prgrph[c * BL:(c + 1) * BL]                      # [8, S, L]
    tokens = np.ascontiguousarray(pr.transpose(1, 0, 2)).reshape(-1)
    idx = np.ascontiguousarray(tokens.reshape(NTILES, 128).T).astype(np.int32)
    kl = keys[c * BL:(c + 1) * BL]                        # [8, K, D]
    kt = np.ascontiguousarray(kl.transpose(2, 0, 1)).reshape(D, BK)
    kt = kt.astype(np.float32)
    m = sent_mask[c * BL:(c + 1) * BL]                    # [8, S] bool
    rminv = np.where(m, np.float32(1.0), np.float32(MINV))
    rminv = np.ascontiguousarray(rminv.T).reshape(1, NSENT)  # [1, t*8+b]
    return {
        "idx": idx,
        "kt": kt,
        "kt16": kt.astype(np.float16),
        "rminv": rminv.astype(np.float32),
    }


def kernel(prgrph, sent_mask, keys, embedding_matrix, fc1_w, fc1_b, fc2_w,
           fc2_b, U, V, W, _trace=False):
    prgrph = np.asarray(prgrph).astype(np.int32)
    sent_mask = np.asarray(sent_mask).astype(bool)
    keys = np.asarray(keys, dtype=np.float32)
    embc = np.ascontiguousarray(
        np.asarray(embedding_matrix, dtype=np.float32)
    ).astype(ml_dtypes.bfloat16)
    f1t = np.ascontiguousarray(np.asarray(fc1_w, dtype=np.float32).T)
    f2w = np.ascontiguousarray(np.asarray(fc2_w, dtype=np.float32))
    f1b = np.asarray(fc1_b, dtype=np.float32).reshape(D, 1).copy()
    f2b = np.asarray(fc2_b, dtype=np.float32).reshape(D, 1).copy()
    u16 = np.ascontiguousarray(np.asarray(U, dtype=np.float32)).astype(
        np.float16)
    v_w = np.ascontiguousarray(np.asarray(V, dtype=np.float32))
    w_w = np.ascontiguousarray(np.asarray(W, dtype=np.float32))
    ones4 = np.repeat(np.eye(4, dtype=np.float32), 32, axis=0)
    ones4 = np.ascontiguousarray(ones4).astype(ml_dtypes.bfloat16)
    id16 = np.eye(D, dtype=np.float16)
    ocol16 = np.ones((D, 1), dtype=np.float16)
    orow16 = np.ones((1, D), dtype=np.float16)

    nc = build_bass()
    common = {
        "embc": embc, "u16": u16, "id16": id16, "ocol16": ocol16,
        "orow16": orow16, "ones4": ones4, "v_w": v_w, "w_w": w_w,
        "f1t": f1t, "f2w": f2w, "f1b": f1b, "f2b": f2b,
    }
    in_maps = []
    for c in range(NCORES):
        im = _prep_core(c, prgrph, sent_mask, keys)
        im.update(common)
        in_maps.append(im)

    res = run_bass_kernel_spmd(nc, in_maps, core_ids=list(range(NCORES)),
                               trace=_trace)
    out = np.empty((B, K, D), dtype=np.float32)
    for c in range(NCORES):
        hh = res.results[c]["out"]                         # [D, BK]
        out[c * BL:(c + 1) * BL] = np.ascontiguousarray(hh.T).reshape(BL, K, D)
    if _trace:
        kernel.last_exec_time_ns = res.exec_time_ns
        kernel.last_trace = res.instructions_and_trace
    return out
